# revision 3
# baseline (speedup 1.0000x reference)
"""Single-head causal attention on 8 TRN2 NeuronCores (Bass/Tile).

Problem: x[B=4,T=4096,E=1024] fp32; Wq/Wk/Wv [E,64]. out = softmax(causal(QK^T/8)) V.

Sharding: core i = (batch b=i//2, parity p=i%2). Each core computes the output
rows for the 256-token blocks of batch b with block index ≡ p (mod 2) — this
balances causal attention work exactly across the two cores of a batch while
keeping one uniform SPMD program; all per-core variation is input data.

Device layout per core (host marshals):
  xt   [1024, T]  x[b].T with columns permuted: own 256-blocks first
                  (ascending), then other-parity blocks.
  wkv  [1024,128] Wk ‖ Wv.
  wq   [1024, 64]
  dtab [128, 4]   causal-mask thresholds for the 4 "tail" k-tiles of each
                  q-span (replicated down partitions).
  out  [T/2, 64]  own q rows in shuffled order.

Algorithm on core: K^T,V^T projected packed (PSUM-accumulated over 8 E-chunks,
fp32r matmuls); V^T transposed to V-natural via PE; Q^T projected for own
tokens. Attention per 256-query span: S^T[k,q] tiles (keys on partitions) so
softmax needs no cross-partition reduce; exp on ACT with no max subtraction
(|score| ≤ 3.5 for this problem's data — validated); causal mask applied only
to the 4 diagonal-region tiles via (iota >= D) * P on DVE with per-core D;
P^T @ [V|1] accumulates O^T and the softmax denominator in one PSUM group.
"""

import os
import numpy as np

import concourse.bass as bass
import concourse.tile as tile
from concourse import bacc, bass_utils, mybir
from concourse.masks import make_identity

F32 = mybir.dt.float32
F32R = mybir.dt.float32r
AF = mybir.ActivationFunctionType
ALU = mybir.AluOpType

B, T_FULL, E, H = 4, 4096, 1024, 64
NCORES = 8
SCALE = float(H) ** -0.5


def r(ap):
    return ap.bitcast(F32R)


def build_program(T, bf16=False):
    """One uniform SPMD program for T tokens per core (T/2 own queries)."""
    IDT = mybir.dt.bfloat16 if bf16 else F32R
    EC = E // 128          # 8 E-chunks
    NT = T // 512          # 512-token tiles
    K128 = T // 128        # total 128-key tiles
    K2 = K128 // 2         # start of other-parity region
    S = T // 512           # q-spans of 256 own tokens  (T/2 own / 256)

    nc = bacc.Bacc(
        "TRN2", target_bir_lowering=False, debug=False, num_devices=NCORES
    )
    xt_d = nc.dram_tensor("xt", [E, T], IDT, kind="ExternalInput")
    wkv_d = nc.dram_tensor("wkv", [E, 2 * H], IDT, kind="ExternalInput")
    wq_d = nc.dram_tensor("wq", [E, H], IDT, kind="ExternalInput")
    dtab_d = nc.dram_tensor("dtab", [128, 4], F32R, kind="ExternalInput")
    out_d = nc.dram_tensor("out", [T // 2, H], F32, kind="ExternalOutput")

    with tile.TileContext(nc) as tc:
        with (
            tc.tile_pool(name="persist", bufs=1) as pp,
            tc.tile_pool(name="stage", bufs=3) as sp,
            tc.tile_pool(name="ppool", bufs=4) as ptp,
            tc.tile_pool(name="opool", bufs=2) as osp,
        ):
            # ---- persistent SBUF ----
            xt = [pp.tile([128, EC, 512], IDT, tag=f"xt{t}", name=f"xt{t}") for t in range(NT)]
            kt = pp.tile([64, T], F32R, tag="kt")
            vb = pp.tile([128, K128, H + 1], F32R, tag="vb")
            qt = pp.tile([64, S, 256], F32R, tag="qt")
            wkv = pp.tile([128, EC, 2 * H], IDT, tag="wkv")
            wq = pp.tile([128, EC, H], IDT, tag="wq")
            dtab = pp.tile([128, 4], F32R, tag="dtab")
            iota = pp.tile([128, 256], F32R, tag="iota")
            iota_i = pp.tile([128, 256], mybir.dt.int32, tag="iota_i")
            ident = pp.tile([128, 128], F32, tag="ident")

            # ---- constants / small inputs ----
            nc.sync.dma_start(
                wkv, wkv_d.ap().rearrange("(c p) m -> p c m", p=128)
            )
            nc.sync.dma_start(wq, wq_d.ap().rearrange("(c p) m -> p c m", p=128))
            nc.sync.dma_start(dtab, dtab_d.ap())
            make_identity(nc, ident)
            nc.gpsimd.iota(
                iota_i,
                pattern=[[1, 256]],
                base=0,
                channel_multiplier=-1,
            )
            nc.vector.tensor_copy(iota, iota_i)
            nc.vector.memset(vb[:, :, H : H + 1].bitcast(mybir.dt.uint32), 0x3F800000)

            # ---- stream x^T in 512-token tiles ----
            xsrc = xt_d.ap().rearrange("(c p) (n t) -> p c n t", p=128, t=512)
            for t in range(NT):
                nc.sync.dma_start(xt[t], xsrc[:, :, t, :])

            order = []
            for g in range(NT // 2):
                order += [g, NT // 2 + g]

            with (
                tc.tile_pool(name="kvpsum", bufs=2, space="PSUM") as kvp,
                tc.tile_pool(name="vtpsum", bufs=1, space="PSUM") as vtp,
                tc.tile_pool(name="qpsum", bufs=1, space="PSUM") as qp,
                tc.tile_pool(name="spsum", bufs=2, space="PSUM") as ssp,
                tc.tile_pool(name="otpsum", bufs=1, space="PSUM") as otp,
                tc.tile_pool(name="trpsum", bufs=1, space="PSUM") as trp,
            ):
                def kv_proj(t):
                    acc = kvp.tile([128, 512], F32, tag="kv")
                    for c in range(EC):
                        nc.tensor.matmul(
                            acc,
                            wkv[:, c, :],
                            xt[t][:, c, :],
                            start=(c == 0),
                            stop=(c == EC - 1),
                        )
                    kvs = sp.tile([128, 512], F32, tag="kvs")
                    nc.vector.tensor_copy(kvs, acc)
                    nc.vector.tensor_copy(
                        kt[:, 512 * t : 512 * (t + 1)], kvs[0:64, :]
                    )
                    for j in range(4):
                        vtr = vtp.tile([128, H], F32, tag="vtr")
                        nc.tensor.transpose(
                            vtr,
                            kvs[64:128, 128 * j : 128 * (j + 1)],
                            ident[64:128, 64:128],
                        )
                        nc.vector.tensor_copy(vb[:, 4 * t + j, 0:H], vtr)

                def q_proj(s):
                    acc = qp.tile([64, 256], F32, tag="qp")
                    for c in range(EC):
                        nc.tensor.matmul(
                            acc,
                            wq[:, c, :],
                            xt[s // 2][:, c, 256 * (s % 2) : 256 * (s % 2 + 1)],
                            start=(c == 0),
                            stop=(c == EC - 1),
                        )
                    nc.vector.tensor_copy(qt[:, s, :], acc)

                def attention(s):
                    tiles = (
                        [(j, -1) for j in range(2 * s)]
                        + [(2 * s, 0), (2 * s + 1, 1)]
                        + [(K2 + j, -1) for j in range(2 * s)]
                        + [(K2 + 2 * s, 2), (K2 + 2 * s + 1, 3)]
                    )
                    ot = otp.tile([H + 1, 256], F32, tag="ot")
                    for i, (j, tail) in enumerate(tiles):
                        spt = ssp.tile([128, 256], F32, tag="s")
                        nc.tensor.matmul(
                            spt,
                            kt[:, 128 * j : 128 * (j + 1)],
                            qt[:, s, :],
                            start=True,
                            stop=True,
                        )
                        pt = ptp.tile([128, 256], F32R, tag="p")
                        nc.scalar.activation(pt, spt, AF.Exp, scale=SCALE)
                        if tail >= 0:
                            ptm = ptp.tile([128, 256], F32R, tag="pm")
                            nc.vector.scalar_tensor_tensor(
                                ptm,
                                iota,
                                dtab[:, tail : tail + 1],
                                pt,
                                ALU.is_ge,
                                ALU.mult,
                            )
                            pt = ptm
                        nc.tensor.matmul(
                            ot,
                            vb[:, j, :],
                            pt,
                            start=(i == 0),
                            stop=(i == len(tiles) - 1),
                        )
                    ots = osp.tile([H + 1, 256], F32, tag="ots")
                    nc.vector.tensor_copy(ots, ot)
                    for hh in range(2):
                        tr = trp.tile([128, H + 1], F32, tag="tr")
                        nc.tensor.transpose(
                            tr,
                            ots[:, 128 * hh : 128 * (hh + 1)],
                            ident[0 : H + 1, 0 : H + 1],
                        )
                        rl = osp.tile([128, 1], F32, tag="rl")
                        nc.vector.reciprocal(rl, tr[:, H : H + 1])
                        ob = osp.tile([128, H], F32, tag="ob")
                        nc.vector.tensor_scalar_mul(ob, tr[:, 0:H], rl)
                        nc.sync.dma_start(
                            out_d.ap()[256 * s + 128 * hh : 256 * s + 128 * (hh + 1), :],
                            ob,
                        )

                for g in range(NT // 2):
                    kv_proj(order[2 * g])
                    kv_proj(order[2 * g + 1])
                    q_proj(2 * g)
                    q_proj(2 * g + 1)
                    attention(2 * g)
                    attention(2 * g + 1)

    nc.compile()
    return nc


def make_in_maps(x, Wk, Wq, Wv, T, bf16=False):
    """Per-core input dicts. x already [B, T, E] fp32 (np)."""
    import ml_dtypes
    idt = ml_dtypes.bfloat16 if bf16 else np.float32
    wkv = np.ascontiguousarray(np.concatenate([Wk, Wv], axis=1))
    in_maps = []
    NB = T // 256
    for core in range(NCORES):
        b, p = core // 2, core % 2
        blocks = list(range(p, NB, 2)) + list(range(1 - p, NB, 2))
        cols = np.concatenate(
            [np.arange(256 * blk, 256 * (blk + 1)) for blk in blocks]
        )
        xt = np.ascontiguousarray(x[b].T[:, cols])
        d23 = [256.0, 384.0] if p == 0 else [-256.0, -128.0]
        dtab = np.tile(
            np.array([[0.0, 128.0, d23[0], d23[1]]], np.float32), (128, 1)
        )
        in_maps.append(
            {
                "xt": xt.astype(idt),
                "wkv": wkv.astype(idt),
                "wq": np.ascontiguousarray(Wq).astype(idt),
                "dtab": dtab,
            }
        )
    return in_maps


def gather_out(results, T):
    """results: list of per-core {name: array}. Returns [B, T, H]."""
    out = np.empty((B, T, H), np.float32)
    NB = T // 256
    for core in range(NCORES):
        b, p = core // 2, core % 2
        o = results[core]["out"]
        own = list(range(p, NB, 2))
        for i, blk in enumerate(own):
            out[b, 256 * blk : 256 * (blk + 1), :] = o[256 * i : 256 * (i + 1), :]
    return out


_CACHE = {}


def _run_pjrt(nc, in_maps, bench_iters=0):
    """Run the SPMD program via PJRT (axon). Optionally time repeated execs.

    Returns (results_per_core, exec_ns_estimate_or_None).
    """
    import time
    import jax
    from jax.sharding import Mesh, PartitionSpec
    from jax.experimental.shard_map import shard_map
    from concourse import bass2jax, mybir as mb

    bass2jax.install_neuronx_cc_hook()
    partition_name = nc.partition_id_tensor.name if nc.partition_id_tensor else None
    in_names, out_names, out_avals, zero_outs = [], [], [], []
    for alloc in nc.m.functions[0].allocations:
        if not isinstance(alloc, mb.MemoryLocationSet):
            continue
        name = alloc.memorylocations[0].name
        if alloc.kind == "ExternalInput":
            if name != partition_name:
                in_names.append(name)
        elif alloc.kind == "ExternalOutput":
            out_names.append(name)
            shape = tuple(alloc.tensor_shape)
            dtype = mb.dt.np(alloc.dtype)
            out_avals.append(jax.core.ShapedArray(shape, dtype))
            zero_outs.append(np.zeros(shape, dtype))
    n_params, n_outs = len(in_names), len(out_avals)
    all_in_names = in_names + out_names
    if partition_name is not None:
        all_in_names = all_in_names + [partition_name]
    donate = tuple(range(n_params, n_params + n_outs))

    def _body(*args):
        operands = list(args)
        if partition_name is not None:
            operands.append(bass2jax.partition_id_tensor())
        return tuple(
            bass2jax._bass_exec_p.bind(
                *operands,
                out_avals=tuple(out_avals),
                in_names=tuple(all_in_names),
                out_names=tuple(out_names),
                lowering_input_output_aliases=(),
                sim_require_finite=True,
                sim_require_nnan=True,
                nc=nc,
            )
        )

    n_cores = NCORES
    devices = jax.devices()[:n_cores]
    mesh = Mesh(np.asarray(devices), ("core",))
    sharded = jax.jit(
        shard_map(
            _body,
            mesh=mesh,
            in_specs=(PartitionSpec("core"),) * (n_params + n_outs),
            out_specs=(PartitionSpec("core"),) * n_outs,
            check_rep=False,
        ),
        donate_argnums=donate,
        keep_unused=True,
    )
    concat_in = [
        np.concatenate([np.asarray(in_maps[c][nm]) for c in range(n_cores)], 0)
        for nm in in_names
    ]
    concat_zero = [
        np.zeros((n_cores * z.shape[0], *z.shape[1:]), z.dtype) for z in zero_outs
    ]
    sh = jax.sharding.NamedSharding(mesh, PartitionSpec("core"))
    dev_in = [jax.device_put(a, sh) for a in concat_in]

    out_arrs = sharded(*dev_in, *[jax.device_put(z, sh) for z in concat_zero])
    jax.block_until_ready(out_arrs)

    exec_ns = None
    if bench_iters > 0:
        def timed(n):
            zs = [
                [jax.device_put(z, sh) for z in concat_zero] for _ in range(n)
            ]
            jax.block_until_ready(zs)
            t0 = time.perf_counter()
            rs = [sharded(*dev_in, *zs[i]) for i in range(n)]
            jax.block_until_ready(rs)
            return time.perf_counter() - t0

        timed(1)
        n_hi = bench_iters
        t1 = min(timed(1) for _ in range(3))
        thi = min(timed(n_hi) for _ in range(3))
        exec_ns = (thi - t1) / (n_hi - 1) * 1e9
        _run_pjrt.t1 = t1
        _run_pjrt.thi = thi

    results = [
        {
            nm: np.asarray(out_arrs[i]).reshape(n_cores, *out_avals[i].shape)[c]
            for i, nm in enumerate(out_names)
        }
        for c in range(n_cores)
    ]
    return results, exec_ns


def kernel(x, Wk, Wq, Wv):
    x = np.asarray(x, np.float32)
    Wk = np.asarray(Wk, np.float32)
    Wq = np.asarray(Wq, np.float32)
    Wv = np.asarray(Wv, np.float32)
    T = x.shape[1]
    bf16 = os.environ.get("KERNEL_BF16", "1") == "1"
    key = (T, bf16)
    if key not in _CACHE:
        _CACHE[key] = build_program(T, bf16=bf16)
    nc = _CACHE[key]
    in_maps = make_in_maps(x, Wk, Wq, Wv, T, bf16=bf16)
    trace = os.environ.get("KERNEL_TRACE", "0") == "1"
    tdir = None
    if trace:
        tdir = os.environ.get("KERNEL_TRACE_DIR") or None
        if tdir:
            kernel.ncall = getattr(kernel, "ncall", -1) + 1
            tdir = os.path.join(tdir, f"call{kernel.ncall}")
            os.makedirs(tdir, exist_ok=True)
    res = bass_utils.run_bass_kernel_spmd(
        nc, in_maps, core_ids=list(range(NCORES)), trace=trace, tmpdir=tdir
    )
    kernel.exec_ns = res.exec_time_ns
    kernel.last_res = res
    return gather_out(res.results, T)



# revision 9
# speedup vs baseline: 1.4364x; 1.4364x over previous
"""Single-head causal attention on 8 TRN2 NeuronCores (Bass/Tile).

Problem: x[B=4,T=4096,E=1024] fp32; Wq/Wk/Wv [E,64]. out = softmax(causal(QK^T/8)) V.

Sharding: core i = (batch b=i//2, parity p=i%2). Each core computes the output
rows for the 256-token blocks of batch b with block index ≡ p (mod 2); one
uniform SPMD program, all per-core variation is input data.

v2 datapath (all-bf16 matmuls, 512-query spans):
  K^T,V^T projected packed in bf16 (PSUM fp32 acc over 8 E-chunks); V^T
  transposed to V-natural via PE in bf16. Q^T projected per 512-token span.
  Scores S^T[k,q] as [128,512] tiles; exp on ACT over paired 2-bank PSUM
  reads [128,1024] -> bf16 P (no max subtraction; |score*scale| <= ~3.6).
  Causal masks via (iota >= D) * P on DVE: own-parity tails use iota1=c-ch
  with constant D in {0,128,256,384}; other-parity tails use iota2=c with
  per-core D from dtab. P^T @ [V|1] accumulates O^T + softmax denominator in
  one PSUM group per span. Epilogue: reciprocal of den row, GpSimd
  partition-broadcast, DVE multiply; out stored transposed [H, T/2] (host
  gather transposes). KV projection of later tiles is interleaved between
  attention pairs to keep the PE busy while ACT drains.
"""

import os
import numpy as np

import concourse.bass as bass
import concourse.tile as tile
from concourse import bacc, bass_utils, mybir
from concourse.masks import make_identity

F32 = mybir.dt.float32
BF16 = mybir.dt.bfloat16
AF = mybir.ActivationFunctionType
ALU = mybir.AluOpType

B, T_FULL, E, H = 4, 4096, 1024, 64
NCORES = 8
SCALE = float(H) ** -0.5


def build_program(T):
    EC = E // 128            # 8 E-chunks
    NT = T // 512            # 8 x^T tiles (0-3 own tokens, 4-7 other)
    NSP = T // 1024          # 4 spans of 512 own queries
    NKT = T // 128           # 32 total 128-key tiles
    KO = NKT // 2            # 16 own k-tiles (kt col offset T//2 for other)

    nc = bacc.Bacc(
        "TRN2", target_bir_lowering=False, debug=False, num_devices=NCORES
    )
    xt_d = nc.dram_tensor("xt", [E, T], BF16, kind="ExternalInput")
    wkv_d = nc.dram_tensor("wkv", [E, 2 * H], BF16, kind="ExternalInput")
    wq_d = nc.dram_tensor("wq", [E, H], BF16, kind="ExternalInput")
    dtab_d = nc.dram_tensor("dtab", [128, 8], F32, kind="ExternalInput")
    out_d = nc.dram_tensor("out", [H, T // 2], F32, kind="ExternalOutput")

    with tile.TileContext(nc) as tc:
        with (
            tc.tile_pool(name="persist", bufs=1) as pp,
            tc.tile_pool(name="stage", bufs=3) as sp,
            tc.tile_pool(name="ppool", bufs=4) as ptp,
            tc.tile_pool(name="rdp", bufs=2) as rdp,
            tc.tile_pool(name="rbp", bufs=2) as rbp,
            tc.tile_pool(name="obp", bufs=2) as obp,
        ):
            # ---- persistent SBUF ----
            xt = [pp.tile([128, EC, 512], BF16, tag=f"xt{t}", name=f"xt{t}") for t in range(NT)]
            kt = pp.tile([64, T], BF16, tag="kt")
            vb = pp.tile([128, NKT, H + 1], BF16, tag="vb")
            qt = pp.tile([64, NSP, 512], BF16, tag="qt")
            wkv = pp.tile([128, EC, 2 * H], BF16, tag="wkv")
            wq = pp.tile([128, EC, H], BF16, tag="wq")
            dtab = pp.tile([128, 8], F32, tag="dtab")
            iota1 = pp.tile([128, 512], F32, tag="iota1")
            iota2 = pp.tile([128, 512], F32, tag="iota2")
            ioti1 = pp.tile([128, 512], mybir.dt.int32, tag="ioti1")
            ioti2 = pp.tile([128, 512], mybir.dt.int32, tag="ioti2")
            identb = pp.tile([128, 128], BF16, tag="identb")

            # ---- constants / small inputs ----
            nc.sync.dma_start(
                wkv, wkv_d.ap().rearrange("(c p) m -> p c m", p=128)
            )
            nc.sync.dma_start(wq, wq_d.ap().rearrange("(c p) m -> p c m", p=128))
            nc.sync.dma_start(dtab, dtab_d.ap())
            make_identity(nc, identb)
            nc.gpsimd.iota(ioti1, pattern=[[1, 512]], base=0, channel_multiplier=-1)
            nc.gpsimd.iota(ioti2, pattern=[[1, 512]], base=0, channel_multiplier=0)
            nc.vector.tensor_copy(iota1, ioti1)
            nc.vector.tensor_copy(iota2, ioti2)
            nc.vector.memset(
                vb[:, :, H : H + 1].bitcast(mybir.dt.uint16), 0x3F80
            )

            # ---- stream x^T, in need order, 2 chunk-halves per tile ----
            xsrc = xt_d.ap().rearrange("(c p) (n t) -> p c n t", p=128, t=512)
            for t in [x for pair in zip(range(NSP), range(NSP, NT)) for x in pair]:
                nc.sync.dma_start(xt[t][:, 0:4, :], xsrc[:, 0:4, t, :])
                nc.sync.dma_start(xt[t][:, 4:8, :], xsrc[:, 4:8, t, :])

            with (
                tc.tile_pool(name="spsum", bufs=2, space="PSUM") as ssp,
                tc.tile_pool(name="opsum", bufs=1, space="PSUM") as otp,
                tc.tile_pool(name="kvpsum", bufs=1, space="PSUM") as kvp,
                tc.tile_pool(name="miscpsum", bufs=1, space="PSUM") as mp,
            ):
                def kv_proj(t):
                    acc = kvp.tile([128, 512], F32, tag="kv")
                    for c in range(EC):
                        nc.tensor.matmul(
                            acc,
                            wkv[:, c, :],
                            xt[t][:, c, :],
                            start=(c == 0),
                            stop=(c == EC - 1),
                        )
                    kvs = sp.tile([128, 512], BF16, tag="kvs")
                    nc.vector.tensor_copy(kvs, acc)
                    nc.vector.tensor_copy(
                        kt[:, 512 * t : 512 * (t + 1)], kvs[0:64, :]
                    )
                    vg = mp.tile([128, 4, H], BF16, tag="vg")
                    for j in range(4):
                        nc.tensor.transpose(
                            vg[:, j, :],
                            kvs[64:128, 128 * j : 128 * (j + 1)],
                            identb[64:128, 64:128],
                        )
                    nc.vector.tensor_copy(vb[:, 4 * t : 4 * t + 4, 0:H], vg)

                def q_proj(s):
                    qacc = mp.tile([64, 512], F32, tag="qp")
                    for c in range(EC):
                        nc.tensor.matmul(
                            qacc,
                            wq[:, c, :],
                            xt[s][:, c, :],
                            start=(c == 0),
                            stop=(c == EC - 1),
                        )
                    nc.vector.tensor_copy(qt[:, s, :], qacc)

                def attention(s, kv_inline):
                    # k-tile list: (region_base_col, j, mask) where mask is
                    # None | (iota, dtab_col)
                    nk = 4 * s + 4
                    tiles = []
                    for j in range(nk):
                        m = (iota1, j - 4 * s) if j >= 4 * s else None
                        tiles.append((0, j, m))
                    for j in range(nk):
                        m = (iota2, 4 + j - 4 * s) if j >= 4 * s else None
                        tiles.append((T // 2, j, m))
                    pairs = [tiles[i : i + 2] for i in range(0, len(tiles), 2)]
                    # spread inline kv work across the span
                    kv_at = {}
                    for i, t in enumerate(kv_inline):
                        kv_at[(i + 1) * len(pairs) // (len(kv_inline) + 1)] = t

                    ot = otp.tile([H + 1, 512], F32, tag="ot")
                    for pi, pair in enumerate(pairs):
                        if pi in kv_at:
                            kv_proj(kv_at[pi])
                        st = ssp.tile([128, 1024], F32, tag="s")
                        for h, (base, j, m) in enumerate(pair):
                            kc = base + 128 * j
                            nc.tensor.matmul(
                                st[:, 512 * h : 512 * (h + 1)],
                                kt[:, kc : kc + 128],
                                qt[:, s, :],
                                start=True,
                                stop=True,
                            )
                        pt = ptp.tile([128, 1024], BF16, tag="pt")
                        nc.scalar.activation(pt, st, AF.Exp, scale=SCALE)
                        if pair[0][2] is not None or pair[1][2] is not None:
                            ptm = ptp.tile([128, 1024], BF16, tag="ptm")
                            for h, (base, j, m) in enumerate(pair):
                                io, dcol = m
                                nc.vector.scalar_tensor_tensor(
                                    ptm[:, 512 * h : 512 * (h + 1)],
                                    io,
                                    dtab[:, dcol : dcol + 1],
                                    pt[:, 512 * h : 512 * (h + 1)],
                                    ALU.is_ge,
                                    ALU.mult,
                                )
                            pt = ptm
                        for h, (base, j, m) in enumerate(pair):
                            vi = j if base == 0 else KO + j
                            nc.tensor.matmul(
                                ot,
                                vb[:, vi, :],
                                pt[:, 512 * h : 512 * (h + 1)],
                                start=(pi == 0 and h == 0),
                                stop=(pi == len(pairs) - 1 and h == 1),
                            )
                    # epilogue: out^T[h,q] = O^T[h,q] / den[q]
                    rden = rdp.tile([1, 512], F32, tag="rd")
                    nc.vector.reciprocal(rden, ot[H : H + 1, :])
                    rb = rbp.tile([H, 512], F32, tag="rb")
                    nc.gpsimd.partition_broadcast(rb, rden)
                    ob = obp.tile([H, 512], F32, tag="ob")
                    nc.vector.scalar_tensor_tensor(
                        ob, ot[0:H, :], 1.0, rb, ALU.mult, ALU.mult
                    )
                    nc.sync.dma_start(out_d.ap()[:, 512 * s : 512 * (s + 1)], ob)

                kv_proj(0)
                kv_proj(NSP)
                q_proj(0)
                for s in range(NSP):
                    nxt = [s + 1, NSP + s + 1] if s + 1 < NSP else []
                    attention(s, nxt)
                    if s + 1 < NSP:
                        q_proj(s + 1)

    nc.compile()
    return nc


def make_in_maps(x, Wk, Wq, Wv, T):
    """Per-core input dicts. x already [B, T, E] fp32 (np)."""
    import ml_dtypes

    wkv = np.ascontiguousarray(np.concatenate([Wk, Wv], axis=1))
    in_maps = []
    NB = T // 256
    for core in range(NCORES):
        b, p = core // 2, core % 2
        blocks = list(range(p, NB, 2)) + list(range(1 - p, NB, 2))
        cols = np.concatenate(
            [np.arange(256 * blk, 256 * (blk + 1)) for blk in blocks]
        )
        xt = np.ascontiguousarray(x[b].T[:, cols])
        oth = [256.0, 256.0, 1e9, 1e9] if p == 0 else [-1e9, -1e9, 256.0, 256.0]
        dtab = np.tile(
            np.array([[0.0, 128.0, 256.0, 384.0] + oth], np.float32), (128, 1)
        )
        in_maps.append(
            {
                "xt": xt.astype(ml_dtypes.bfloat16),
                "wkv": wkv.astype(ml_dtypes.bfloat16),
                "wq": np.ascontiguousarray(Wq).astype(ml_dtypes.bfloat16),
                "dtab": dtab,
            }
        )
    return in_maps


def gather_out(results, T):
    """results: list of per-core {name: array}. Returns [B, T, H]."""
    out = np.empty((B, T, H), np.float32)
    NB = T // 256
    for core in range(NCORES):
        b, p = core // 2, core % 2
        o = results[core]["out"]  # [H, T//2]
        own = list(range(p, NB, 2))
        for i, blk in enumerate(own):
            out[b, 256 * blk : 256 * (blk + 1), :] = o[
                :, 256 * i : 256 * (i + 1)
            ].T
    return out


_CACHE = {}


def kernel(x, Wk, Wq, Wv):
    x = np.asarray(x, np.float32)
    Wk = np.asarray(Wk, np.float32)
    Wq = np.asarray(Wq, np.float32)
    Wv = np.asarray(Wv, np.float32)
    T = x.shape[1]
    if T not in _CACHE:
        _CACHE[T] = build_program(T)
    nc = _CACHE[T]
    in_maps = make_in_maps(x, Wk, Wq, Wv, T)
    trace = os.environ.get("KERNEL_TRACE", "0") == "1"
    tdir = None
    if trace:
        tdir = os.environ.get("KERNEL_TRACE_DIR") or None
        if tdir:
            kernel.ncall = getattr(kernel, "ncall", -1) + 1
            tdir = os.path.join(tdir, f"call{kernel.ncall}")
            os.makedirs(tdir, exist_ok=True)
    res = bass_utils.run_bass_kernel_spmd(
        nc, in_maps, core_ids=list(range(NCORES)), trace=trace, tmpdir=tdir
    )
    kernel.exec_ns = res.exec_time_ns
    kernel.last_res = res
    return gather_out(res.results, T)
